# revision 19
# baseline (speedup 1.0000x reference)
"""Trainium2 Bass kernel for the time-binned MoE EmbeddingClassifier.

Model: 11 expert MLPs (1536 -> 3072 -> 3072 -> 5242, exact GELU between
layers, log_softmax output). Each sample is routed to one expert by
bin = trunc((1 - mask_frac) / 0.1).

Strategy (8 NeuronCores, expert-parallel with host-side routing):
  - Routing is computed on the host from mask_frac; samples are grouped by
    expert. Only the routed expert runs per sample (11x less compute than
    the reference's run-all-then-select).
  - Experts 0..7 are whole-expert assigned to cores 0..7.
  - Experts 8 and 9 are each split 4 ways along the hidden dimension
    (cores 0-3 handle expert 8, cores 4-7 handle expert 9): each core
    computes the full layer 1, a 768-column slice of layer 2, and the
    matching 768-row slice of layer 3, producing a full-width partial
    logit sum.
  - Precision: all weights and activations stream as e4m3 fp8 with a x64
    power-of-2 pre-scale (|W|~0.02 sits in e4m3's subnormal range) and
    the descale folded into the PSUM-drain ACT ops; matmuls run in
    DoubleRow perf mode (2 k-tiles per pass). Accumulation stays fp32 in
    PSUM; logits drain to fp16.
  - The device writes RAW fp16 logits chunk-by-chunk as layer-3 chunks
    complete (no device softmax): output writes overlap the remaining
    weight stream instead of piling into a HAM-throttled tail, and the
    host applies bias-3 + log_softmax in fp64 during the gather.
  - Expert 10 (hit only when mask_frac == 0.0 exactly) and any samples
    beyond the per-expert capacity of 128 are computed on the host in
    fp32 as a correctness fallback.

Device layout: activations ride the partition dim as [samples<=128, feat];
weights stream as the moving matmul operand in DoubleRow order. Weights are
host-packed into per-output-chunk column blocks so the k-loop accumulates
into a single PSUM bank with back-to-back matmuls, and each block arrives
via ~0.5 MB DMA pieces so the PE never starves. Layer emission interleaves
the whole-expert and quarter-expert units (wL1, qL1, wL2, qL2, wL3, qL3)
to keep the weight-DMA queue saturated end-to-end and the PE continuously
busy (the PE p-state only reaches 2.4 GHz after ~3 us of uninterrupted
execution). Between layers the activations are transposed 128x128 via the
PE and drained to fp8.
"""

import os
import sys

if "/opt/trn_rl_repo" not in sys.path:
    sys.path.insert(0, "/opt/trn_rl_repo")

import numpy as np
import ml_dtypes

import concourse.bass as bass
import concourse.tile as tile
from concourse import bacc, mybir
from concourse.bass_utils import run_bass_kernel_spmd

HALF = mybir.dt.float16
FP8 = mybir.dt.float8e4
F32 = mybir.dt.float32
AF = mybir.ActivationFunctionType
NHALF = np.float16
NF8 = ml_dtypes.float8_e4m3
FP8_SCALE = 64.0     # power-of-2 pre-scale: |W|~0.02 sits in e4m3's subnormal
                     # range, x64 recenters it; descale rides the ACT op
DRMODE = mybir.MatmulPerfMode.DoubleRow
# knobs for A/B runs
L1_DR = os.environ.get("MOE_L1DR", "1") == "1"   # DoubleRow layer 1 (fp8 x)
WBUFS = int(os.environ.get("MOE_WBUFS", "10"))

E = 11
D = 1536
H = 3072
C = 5242
B = 1024
CAP = 128            # per-expert sample capacity on device
CPAD = 5248          # C padded to a multiple of 128 (10x512 + 128)
CMAIN = 5120         # first 10 layer-3 chunks (512 wide)
NK1 = D // 128       # 12 k-tiles for layer 1
NK2 = H // 128       # 24 k-tiles for layers 2/3
QCOLS = H // 4       # 768-wide hidden slice for the split experts

LAST_RESULTS = None  # BassKernelResults of the most recent run (for test.py)

_NC_CACHE = {}


def _chunk_mm(nc, wpool, pspool, lhs_full, nk, wdram, jrow, cw, npieces,
              name, final_stop=False, dr=True):
    """Accumulate one [128, cw] output chunk over nk k-tiles into one PSUM
    tile. Weight block [128, nk*cw] is DMAed in npieces k-contiguous pieces
    (subtile deps let early matmuls start before the whole block lands)."""
    psum = pspool.tile([128, 512], F32, tag="acc", name=f"ps_{name}")
    wblk = wpool.tile([128, nk * cw], FP8, tag="wblk", name=f"wb_{name}")
    cols = nk * cw
    if npieces == 0:     # warmup split: small leading pieces so the first
        bounds = [0, 2 * cw, 4 * cw, 8 * cw, cols]   # matmuls start early
    else:
        kg = nk // npieces
        bounds = [pc * kg * cw for pc in range(npieces)] + [cols]
    for c0, c1 in zip(bounds, bounds[1:]):
        nc.sync.dma_start(wblk[:, c0:c1],
                          wdram[jrow * 128:(jrow + 1) * 128, c0:c1])
    if dr:
        for t in range(nk // 2):
            lhs = lhs_full[:, 256 * t:256 * (t + 1)].rearrange(
                "p (i m) -> p i m", i=2)
            rhs = wblk[:, 2 * cw * t:2 * cw * (t + 1)].rearrange(
                "p (i n) -> p i n", i=2)
            nc.tensor.matmul(psum[:, :cw], lhs, rhs, perf_mode=DRMODE,
                             start=(t == 0),
                             stop=(final_stop and t == nk // 2 - 1))
    else:
        for k in range(nk):
            nc.tensor.matmul(psum[:, :cw], lhs_full[:, k * 128:(k + 1) * 128],
                             wblk[:, k * cw:(k + 1) * cw],
                             start=(k == 0),
                             stop=(final_stop and k == nk - 1))
    return psum


def _transpose(nc, hpool, tppool, src, ncols, ident_t, name):
    """Transpose src [128, ncols] per 128-chunk -> new fp8 tile (tensor
    engine transpose + DVE drain-copy casting to fp8)."""
    out = hpool.tile([128, H], FP8, tag="ht", name=f"t_{name}")
    for k in range(ncols // 128):
        tp = tppool.tile([128, 128], HALF, tag="tp", name=f"tp_{name}_{k}")
        nc.tensor.transpose(tp[:], src[:, k * 128:(k + 1) * 128], ident_t[:])
        nc.vector.tensor_copy(out[:, k * 128:(k + 1) * 128], tp[:])
    return out


def _layer12(nc, pools, lhs, wdram, bias_t, ones_t, ident_t, nk, ncols, cw,
             warm, name, with_bias, dr, npieces=3):
    """One GELU layer: gelu(lhs.T @ W + b) -> transposed fp8 [128, ncols]."""
    hpool, wpool, pspool, tppool = pools
    h = hpool.tile([128, H], HALF, tag="h", name=f"h_{name}")
    for j in range(ncols // cw):
        np_ = 0 if (warm and j == 0) else npieces
        ps = _chunk_mm(nc, wpool, pspool, lhs, nk, wdram, j, cw, np_,
                       f"{name}j{j}", final_stop=not with_bias, dr=dr)
        if with_bias:
            nc.tensor.matmul(ps[:, :cw], ones_t[:],
                             bias_t[:, j * cw:(j + 1) * cw],
                             start=False, stop=True)
        nc.scalar.activation(h[:, j * cw:(j + 1) * cw], ps[:, :cw], AF.Gelu,
                             scale=1.0 / FP8_SCALE)
    return _transpose(nc, hpool, tppool, h, ncols, ident_t, name)


def _layer3_parts(nc, pools, zpool, h2t, w3cb, w3cbl, out_ap, nk3, name,
                  fused, npieces=3):
    """Layer 3 as a list of emit-closures (5 chunk pairs + the last 128-wide
    chunk) so the caller can interleave the whole/quarter units. Raw logits
    z = h2 @ W3 drain to fp16 and stream out (bias-3 + log_softmax happen
    on the host). Chunk pairs share one output write (2KB per partition
    row); with fused=True a pair's weights arrive as one [128, 2*nk3*512]
    block in two pieces (keeps DMA packets >= 3KB for the narrow
    quarter-unit blocks)."""
    hpool, wpool, pspool, tppool = pools
    blkc = nk3 * 512

    def emit_pair(jp):
        zh = zpool.tile([128, 1024], HALF, tag="zh", name=f"zh_{name}_{jp}")
        if fused:
            wblk = wpool.tile([128, 2 * blkc], FP8, tag="qwblk", bufs=3,
                              name=f"wb_{name}p{jp}")
            nc.sync.dma_start(wblk[:, :blkc],
                              w3cb[jp * 128:(jp + 1) * 128, :blkc])
            nc.sync.dma_start(wblk[:, blkc:],
                              w3cb[jp * 128:(jp + 1) * 128, blkc:])
        for h_ in range(2):
            j = 2 * jp + h_
            if fused:
                psum = pspool.tile([128, 512], F32, tag="acc",
                                   name=f"ps_{name}j{j}")
                for t in range(nk3 // 2):
                    lhs = h2t[:, 256 * t:256 * (t + 1)].rearrange(
                        "p (i m) -> p i m", i=2)
                    rhs = wblk[:, h_ * blkc + 1024 * t:
                               h_ * blkc + 1024 * (t + 1)].rearrange(
                        "p (i n) -> p i n", i=2)
                    nc.tensor.matmul(psum[:, :512], lhs, rhs,
                                     perf_mode=DRMODE, start=(t == 0),
                                     stop=(t == nk3 // 2 - 1))
                ps = psum
            else:
                ps = _chunk_mm(nc, wpool, pspool, h2t, nk3, w3cb, j, 512,
                               npieces, f"{name}j{j}", final_stop=True,
                               dr=True)
            nc.scalar.activation(zh[:, h_ * 512:(h_ + 1) * 512], ps[:, :512],
                                 AF.Copy, bias=0.0, scale=1.0 / FP8_SCALE)
        nc.gpsimd.dma_start(out_ap[:, jp * 1024:(jp + 1) * 1024], zh[:])

    def emit_last():
        ps = _chunk_mm(nc, wpool, pspool, h2t, nk3, w3cbl, 0, 128, 1,
                       f"{name}j10", final_stop=True, dr=True)
        zh = zpool.tile([128, 1024], HALF, tag="zh", name=f"zh_{name}_l")
        nc.scalar.activation(zh[:, :128], ps[:, :128], AF.Copy,
                             bias=0.0, scale=1.0 / FP8_SCALE)
        nc.gpsimd.dma_start(out_ap[:, CMAIN:CPAD], zh[:, :128])

    return [lambda jp=jp: emit_pair(jp) for jp in range(5)] + [emit_last]


def _build_nc(with_bias=True):
    nc = bacc.Bacc("TRN2", target_bir_lowering=False, debug=False,
                   num_devices=8)

    xdt = FP8 if L1_DR else HALF

    def din(name, shape, dt=FP8):
        return nc.dram_tensor(name, shape, dt, kind="ExternalInput").ap()

    xw = din("xw", [128, D], xdt)
    xq = din("xq", [128, D], xdt)
    w1cb = din("w1cb", [6 * 128, NK1 * 512])
    w2cb = din("w2cb", [6 * 128, NK2 * 512])
    w3cb = din("w3cb", [10 * 128, NK2 * 512])
    w3cbl = din("w3cbl", [128, NK2 * 128])
    w1qcb = din("w1qcb", [6 * 128, NK1 * 512])
    w2qcb = din("w2qcb", [2 * 128, NK2 * 384])
    w3qcb = din("w3qcb", [5 * 128, 2 * 6 * 512])
    w3qcbl = din("w3qcbl", [128, 6 * 128])
    if with_bias:
        b1w = din("b1w", [1, H], HALF)
        b2w = din("b2w", [1, H], HALF)
        b1q = din("b1q", [1, H], HALF)
        b2q = din("b2q", [1, QCOLS], HALF)
    ones = din("ones", [1, 128], HALF)
    ident = din("ident", [128, 128], HALF)
    outw = nc.dram_tensor("outw", [128, CPAD], HALF, kind="ExternalOutput").ap()
    outq = nc.dram_tensor("outq", [128, CPAD], HALF, kind="ExternalOutput").ap()

    with tile.TileContext(nc) as tc:
        with tc.tile_pool(name="hp", bufs=2) as hpool, \
             tc.tile_pool(name="ht", bufs=4) as htpool, \
             tc.tile_pool(name="wp", bufs=WBUFS) as wpool, \
             tc.tile_pool(name="zp", bufs=5) as zpool, \
             tc.tile_pool(name="cp", bufs=1) as cpool, \
             tc.tile_pool(name="ps", bufs=5, space="PSUM") as pspool, \
             tc.tile_pool(name="tp", bufs=3, space="PSUM") as tppool:
        # hpool: pre-transpose fp16 h tiles; htpool: transposed fp8 tiles
        # (separate pools so a ring slot never pairs a 6KB and 3KB tile)

            # x first (first matmuls need it, in 3 pieces so matmul 0 can
            # start early), consts on the ACT HWDGE queue so they don't
            # delay the weight-block stream on the SP queue
            xw_t = cpool.tile([128, D], xdt, tag="xw")
            nc.gpsimd.dma_start(xw_t[:], xw)
            ones_t = cpool.tile([1, 128], HALF, tag="ones")
            nc.scalar.dma_start(ones_t[:], ones)
            ident_t = cpool.tile([128, 128], HALF, tag="ident")
            nc.scalar.dma_start(ident_t[:], ident)
            xq_t = cpool.tile([128, D], xdt, tag="xq")
            nc.gpsimd.dma_start(xq_t[:], xq)
            if with_bias:
                b1w_t = cpool.tile([1, H], HALF, tag="b1w")
                nc.scalar.dma_start(b1w_t[:], b1w)
                b2w_t = cpool.tile([1, H], HALF, tag="b2w")
                nc.scalar.dma_start(b2w_t[:], b2w)
                b1q_t = cpool.tile([1, H], HALF, tag="b1q")
                nc.scalar.dma_start(b1q_t[:], b1q)
                b2q_t = cpool.tile([1, QCOLS], HALF, tag="b2q")
                nc.scalar.dma_start(b2q_t[:], b2q)
            else:
                b1w_t = b2w_t = b1q_t = b2q_t = None

            pools = (hpool, wpool, pspool, tppool)
            tpools = (htpool, wpool, pspool, tppool)

            # layer-interleaved emission: the DMA stream stays saturated and
            # each unit's layer boundary (transpose barrier) is hidden
            # behind the other unit's matmuls
            h1t_w = _layer12(nc, pools, xw_t[:], w1cb, b1w_t, ones_t,
                             ident_t, NK1, H, 512, True, "wl1", with_bias,
                             dr=L1_DR, npieces=2)
            h1t_q = _layer12(nc, pools, xq_t[:], w1qcb, b1q_t, ones_t,
                             ident_t, NK1, H, 512, False, "ql1", with_bias,
                             dr=L1_DR, npieces=2)
            h2t_w = _layer12(nc, tpools, h1t_w[:], w2cb, b2w_t, ones_t,
                             ident_t, NK2, H, 512, False, "wl2", with_bias,
                             dr=True)
            h2t_q = _layer12(nc, tpools, h1t_q[:], w2qcb, b2q_t, ones_t,
                             ident_t, NK2, QCOLS, 384, False, "ql2",
                             with_bias, dr=True, npieces=2)
            # interleave the heavy whole-unit L3 chunks (24 DR matmuls per
            # pair) with the light quarter-unit pairs (6) so PE load and
            # DMA block sizes stay mixed and output writes spread out
            parts_w = _layer3_parts(nc, tpools, zpool, h2t_w, w3cb, w3cbl,
                                    outw, NK2, "wl3", fused=False)
            parts_q = _layer3_parts(nc, tpools, zpool, h2t_q, w3qcb, w3qcbl,
                                    outq, QCOLS // 128, "ql3", fused=True)
            for pw, pq in zip(parts_w, parts_q):
                pw()
                pq()
    nc.compile()
    return nc


def _cb_pack(W, cw, dr=True):
    """[K, Ctot] -> per-cw-chunk column blocks [nch*128, nk*cw] where
    block row p, col k*cw + c = W[k*128 + p, j*cw + c] * FP8_SCALE (fp8).
    In DoubleRow order rows pair up per 256-super: col t*2cw + i*cw + c
    maps to row 256t + 128i + p."""
    K, Ct = W.shape
    nk, nch = K // 128, Ct // cw
    Wr = (np.asarray(W, dtype=np.float32) * FP8_SCALE).astype(NF8)
    Wr = Wr.reshape(nk, 128, Ct)
    out = np.empty((nch * 128, nk * cw), dtype=NF8)
    for j in range(nch):
        blk = Wr[:, :, j * cw:(j + 1) * cw]        # [nk, 128, cw]
        if dr:
            # [t, i, p, c] -> [p, t, i, c] -> cols ordered t*2cw + i*cw + c
            out[j * 128:(j + 1) * 128] = (
                blk.reshape(nk // 2, 2, 128, cw).transpose(2, 0, 1, 3)
                .reshape(128, nk * cw))
        else:
            out[j * 128:(j + 1) * 128] = (
                blk.transpose(1, 0, 2).reshape(128, nk * cw))
    return out


def _erf(v):
    try:
        from scipy.special import erf
        return erf(v)
    except ImportError:
        import math
        return np.vectorize(math.erf)(v)


def _host_expert(x_rows, W1e, b1e, W2e, b2e, W3e, b3e):
    """fp32 numpy fallback, mirroring the reference exactly."""

    def gelu(v):
        return (v * 0.5 * (1.0 + _erf(v / np.sqrt(2.0)))).astype(np.float32)

    h1 = gelu(x_rows @ W1e + b1e)
    h2 = gelu(h1 @ W2e + b2e)
    z = (h2 @ W3e + b3e).astype(np.float64)
    return _log_softmax(z)


def _log_softmax(z):
    m = z.max(axis=1, keepdims=True)
    lse = np.log(np.exp(z - m).sum(axis=1, keepdims=True)) + m
    return (z - lse).astype(np.float32)


def kernel(x, mask_frac, W1, b1, W2, b2, W3, b3):
    global LAST_RESULTS, _NC_CACHE

    x = np.asarray(x, dtype=np.float32)
    mask_frac = np.asarray(mask_frac, dtype=np.float32)
    W1 = np.asarray(W1, dtype=np.float32)
    b1 = np.asarray(b1, dtype=np.float32)
    W2 = np.asarray(W2, dtype=np.float32)
    b2 = np.asarray(b2, dtype=np.float32)
    W3 = np.asarray(W3, dtype=np.float32)
    b3 = np.asarray(b3, dtype=np.float32)

    # host routing, mirroring the reference's fp32 arithmetic
    t = np.float32(1.0) - mask_frac
    bins = (t / np.float32(0.1)).astype(np.int32)

    with_bias = bool(b1.any() or b2.any())

    groups = [np.where(bins == e)[0] for e in range(E)]
    fallback = []  # (expert, sample indices) pairs computed on host
    dev_groups = []
    for e in range(10):
        idx = groups[e]
        if len(idx) > CAP:
            fallback.append((e, idx[CAP:]))
            idx = idx[:CAP]
        dev_groups.append(idx)
    if len(groups[10]):
        fallback.append((10, groups[10]))

    nxdt = NF8 if L1_DR else NHALF

    def pack_x(idx):
        # [128, D] with xs[p, k*128 + n] = x[idx[n], k*128 + p]
        xt = np.zeros((128, D), dtype=nxdt)
        if len(idx):
            xe = x[idx].astype(nxdt)            # [n, D]
            xr = np.ascontiguousarray(
                xe.reshape(len(idx), NK1, 128).transpose(2, 1, 0))
            xt.reshape(128, NK1, 128)[:, :, :len(idx)] = xr
        return xt

    ones_np = np.ones((1, 128), dtype=NHALF)
    ident_np = np.eye(128, dtype=NHALF)

    in_maps = []
    for c in range(8):
        q = 8 if c < 4 else 9          # split expert handled by this core
        qq = c % 4                     # hidden-dim quarter index
        w3pad = np.zeros((H, CPAD), dtype=np.float32)
        w3pad[:, :C] = W3[c]
        w3qpad = np.zeros((QCOLS, CPAD), dtype=np.float32)
        w3qpad[:, :C] = W3[q][qq * QCOLS:(qq + 1) * QCOLS]
        bias_ins = {
            "b1w": (b1[c] * FP8_SCALE).astype(NHALF).reshape(1, H),
            "b2w": (b2[c] * FP8_SCALE).astype(NHALF).reshape(1, H),
            "b1q": (b1[q] * FP8_SCALE).astype(NHALF).reshape(1, H),
            "b2q": np.ascontiguousarray(
                (b2[q][qq * QCOLS:(qq + 1) * QCOLS] * FP8_SCALE)
                .astype(NHALF)).reshape(1, QCOLS),
        } if with_bias else {}
        in_maps.append({
            **bias_ins,
            "xw": pack_x(dev_groups[c]),
            "xq": pack_x(dev_groups[q]),
            "w1cb": _cb_pack(W1[c], 512, dr=L1_DR),
            "w2cb": _cb_pack(W2[c], 512),
            "w3cb": _cb_pack(w3pad[:, :CMAIN], 512),
            "w3cbl": _cb_pack(w3pad[:, CMAIN:], 128),
            "w1qcb": _cb_pack(W1[q], 512, dr=L1_DR),
            "w2qcb": _cb_pack(W2[q][:, qq * QCOLS:(qq + 1) * QCOLS], 384),
            # fused pairs: [5*128, 2*3072] with chunk 2jp+h at cols h*3072
            "w3qcb": np.ascontiguousarray(
                _cb_pack(w3qpad[:, :CMAIN], 512)
                .reshape(5, 2, 128, 6 * 512).transpose(0, 2, 1, 3)
                .reshape(5 * 128, 2 * 6 * 512)),
            "w3qcbl": _cb_pack(w3qpad[:, CMAIN:], 128),
            "ones": ones_np,
            "ident": ident_np,
        })

    if with_bias not in _NC_CACHE:
        _NC_CACHE[with_bias] = _build_nc(with_bias)
    res = run_bass_kernel_spmd(_NC_CACHE[with_bias], in_maps,
                               core_ids=list(range(8)))
    LAST_RESULTS = res

    out = np.zeros((B, C), dtype=np.float32)
    # whole experts: raw fp16 logits -> + b3 -> log_softmax (fp64)
    for c in range(8):
        idx = dev_groups[c]
        if len(idx):
            z = res.results[c]["outw"][:len(idx), :C].astype(np.float64)
            out[idx] = _log_softmax(z + b3[c])

    # split experts: host-sum the 4 hidden-quarter partials + b3, log_softmax
    for qe, cores in ((8, (0, 1, 2, 3)), (9, (4, 5, 6, 7))):
        idx = dev_groups[qe]
        if not len(idx):
            continue
        zsum = np.zeros((len(idx), C), dtype=np.float64)
        for c in cores:
            zsum += res.results[c]["outq"][:len(idx), :C]
        out[idx] = _log_softmax(zsum + b3[qe])

    for e, idx in fallback:
        out[idx] = _host_expert(x[idx], W1[e], b1[e], W2[e], b2[e],
                                W3[e], b3[e])
    return out
